# revision 6
# baseline (speedup 1.0000x reference)
"""Locally-connected graph-conv kernel for Trainium2 (Bass/Tile).

Computes out[b,t,m] = sum_n x[b,t,n] * (S*W)[n,m] + bias[m] for
x [64, 2048, 208], W/S [208, 208], bias [208].

The ring-graph support S is a +-4 band (mod 208): output node m only
depends on x nodes m-4..m+4. The 208 outputs are split into FOUR
groups of 52, each needing a 60-row contraction slice, and the four
[60,52] premasked weight tiles are packed into the 2x2 quadrants of
the 128x128 PE array via tile_position:
    G0 outs   0.. 51  rot rows   0.. 59  quadrant (0,0)    x-tile E
    G1 outs  52..103  rot rows  52..111  quadrant (64,64)  x-tile E
    G2 outs 104..155  rot rows 104..163  quadrant (0,64)   x-tile O
    G3 outs 156..207  rot rows 156..215  quadrant (64,0)   x-tile O
(rot row j = node (j-4) mod 208). Per 512 t-columns, FOUR matmuls run
CONCURRENTLY in the four quadrants (hardware per-subarray concurrency;
LDWEIGHTS for one quadrant overlaps in-flight matmuls in others), so
every t-column is streamed once per x-tile instead of once per
104-output block: ~2x the PE throughput of the 2-block layout and,
critically, fast enough (~2.2 us per 2048 cols even at the cold 1.2
GHz HAM clock) that the PE never paces the DMA pipeline - no HAM
warm-up games needed.

The host stacks x into the two quadrant layouts (E: G0 rows at
partitions 0:60, G1 at 64:124; O: G2/G3 likewise; 4 junk rows pad each
60-row group to the 64-partition quadrant boundary). G0/G1 land in the
same PSUM bank at partitions 0:52 and 64:116, so one [116,x] eviction
per bank-pair moves both (junk partitions 52:64 ride along and are
dropped by the host at gather).

Everything that touches HBM is bf16 (PSUM accumulation stays fp32).
Measured HW behavior this build is tuned against:
 - ONE HWDGE ring sustains only ~260 GB/s; the ~360 GB/s HBM rate
   needs both rings pulling. Loads split stream-wise: xE chunks on the
   Sync ring (wh at its head), xO chunks on the Scalar ring (bias at
   its head), strictly front-to-back so the head chunk is never
   starved behind later bytes. Stores go on the GpSimd SWDGE ring - a
   third descriptor stream that round-robins against both load rings
   for write bandwidth without queueing a store behind unrelated load
   bytes (and without eating ScalarE issue time).
 - ScalarE interleaves its per-chunk eviction with the xO load issue
   for chunk c+3: load issue stays 3 chunks ahead of consumption.
 - chunk sizes taper at BOTH ends: small first chunks start compute
   early, small last chunks keep the store tail short.
 - PSUM->SBUF eviction is 1 elem/lane/cycle (fp32 source), so the E
   bank-pair evicts on VectorE and the O bank-pair on ScalarE, both
   fusing bias and the fp32->bf16 down-convert.
The host transposes y^T back at gather.
"""

import numpy as np
import ml_dtypes
from contextlib import ExitStack

import concourse.bacc as bacc
import concourse.mybir as mybir
import concourse.tile as tile
from concourse.bass_utils import run_bass_kernel_spmd

N = 208                      # nodes
K = 4                        # band half-width of S
G = 52                       # output nodes per PE quadrant tile
GR = G + 2 * K               # 60 contraction rows per group
QP = 64                      # quadrant partition pitch
GE = QP + GR                 # 124 used partitions per x tile
EV = QP + G                  # 116 evicted partitions per bank-pair
XROWS = 128                  # x tile partition count (DMA-friendly)
WPAD = 1024                  # wh DRAM row padding (2 KB rows -> fast DMA)
BPAD = 256                   # bias DRAM row padding (1 KB f32 rows)
N_CORES = 8
B, T = 64, 2048
ROWS_TOTAL = B * T           # 131072
SHARD = ROWS_TOTAL // N_CORES    # 16384 rows per core
TB = 512                     # moving-block columns per matmul (fp32 PSUM max)
TB2 = 2 * TB                 # eviction group (2 PSUM banks)
CHUNKS = [1024, 1024, 2048, 2048, 2048, 2048, 2048, 1024, 1024, 1024, 1024]
assert sum(CHUNKS) == SHARD
PREF = 3                     # chunks of xO-load-issue lookahead on ScalarE

FP32 = mybir.dt.float32
BF16 = mybir.dt.bfloat16
NP_BF16 = ml_dtypes.bfloat16
IDENT = mybir.ActivationFunctionType.Identity

_CACHE = {}
LAST_RESULTS = None          # BassKernelResults of the most recent run


def _kernel_body(tc):
    nc = tc.nc
    x_e = nc.dram_tensor("xe", [XROWS, SHARD], BF16, kind="ExternalInput").ap()
    x_o = nc.dram_tensor("xo", [XROWS, SHARD], BF16, kind="ExternalInput").ap()
    w_d = nc.dram_tensor("wh", [XROWS, WPAD], BF16, kind="ExternalInput").ap()
    b_d = nc.dram_tensor("bias", [XROWS, BPAD], FP32, kind="ExternalInput").ap()
    o_d = nc.dram_tensor("outt", [2 * EV, SHARD], BF16, kind="ExternalOutput").ap()

    with ExitStack() as ctx:
        const = ctx.enter_context(tc.tile_pool(name="const", bufs=1))

        # Ring heads: wh leads Sync, bias leads Scalar (both tiny, done
        # in <1 us at the head of their FIFOs).
        wh = const.tile([XROWS, WPAD], BF16, tag="wh")
        nc.sync.dma_start(wh, w_d)
        bt = const.tile([XROWS, BPAD], FP32, tag="bt")
        nc.scalar.dma_start(bt, b_d)
        bAc = bt[0:EV, 0:1]
        bBc = bt[0:EV, 1:2]

        oAp = ctx.enter_context(tc.tile_pool(name="oAp", bufs=3))
        oBp = ctx.enter_context(tc.tile_pool(name="oBp", bufs=3))
        psAp = ctx.enter_context(tc.tile_pool(name="psAp", bufs=2, space="PSUM"))
        psBp = ctx.enter_context(tc.tile_pool(name="psBp", bufs=2, space="PSUM"))

        # persistent x tiles; loads issued chunk-order, xE on Sync.
        # xO on Scalar, interleaved with evictions below (PREF ahead).
        xts = []
        col = 0
        for c, csz in enumerate(CHUNKS):
            xe = const.tile([XROWS, csz], BF16, tag=f"xe_{c}")
            xo = const.tile([XROWS, csz], BF16, tag=f"xo_{c}")
            xts.append((xe, xo, col, csz))
            col += csz

        def issue_loads(c):
            xe, xo, col, csz = xts[c]
            lsl = slice(col, col + csz)
            nc.sync.dma_start(xe, x_e[:, lsl])
            nc.scalar.dma_start(xo, x_o[:, lsl])

        for c in range(PREF):
            issue_loads(c)

        n_chunks = len(CHUNKS)
        for c, (xe, xo, col, csz) in enumerate(xts):
            tsl = slice(col, col + csz)
            oA_t = oAp.tile([EV, csz], BF16, tag="oA")
            oB_t = oBp.tile([EV, csz], BF16, tag="oB")
            for s in range((csz + TB2 - 1) // TB2):
                g0 = s * TB2
                gw = min(TB2, csz - g0)
                g = slice(g0, g0 + gw)
                # [128, 1024] PSUM tiles (2 banks); each of the four
                # quadrant matmuls targets one bank, partitions 0:52 or
                # 64:116.
                psA = psAp.tile([XROWS, TB2], FP32, tag="psA")
                psB = psBp.tile([XROWS, TB2], FP32, tag="psB")
                for q0 in range(0, gw, TB):
                    qs = slice(g0 + q0, g0 + q0 + TB)
                    qp = slice(q0, q0 + TB)
                    nc.tensor.matmul(psA[0:G, qp], wh[0:GR, 0:G],
                                     xe[0:GR, qs], start=True, stop=True,
                                     tile_position=(0, 0))
                    nc.tensor.matmul(psA[QP:EV, qp], wh[QP:GE, 0:G],
                                     xe[QP:GE, qs], start=True, stop=True,
                                     tile_position=(QP, QP))
                    nc.tensor.matmul(psB[QP:EV, qp], wh[0:GR, G : 2 * G],
                                     xo[0:GR, qs], start=True, stop=True,
                                     tile_position=(0, QP))
                    nc.tensor.matmul(psB[0:G, qp], wh[QP:GE, G : 2 * G],
                                     xo[QP:GE, qs], start=True, stop=True,
                                     tile_position=(QP, 0))
                # one [116,gw] eviction per bank-pair moves both groups
                # (junk partitions 52:64 ride along); bias + fp32->bf16
                # fused. E pair on VectorE, O pair on ScalarE.
                nc.vector.tensor_scalar_add(oA_t[:, g], psA[0:EV, 0:gw], bAc)
                nc.scalar.activation(oB_t[:, g], psB[0:EV, 0:gw], IDENT, bias=bBc)
            # next xO load issue lands here in ScalarE program order,
            # keeping issue PREF chunks ahead of consumption.
            if c + PREF < n_chunks:
                issue_loads(c + PREF)
            # stores on the GpSimd SWDGE ring: own FIFO, never behind
            # loads.
            nc.gpsimd.dma_start(o_d[0:EV, tsl], oA_t)
            nc.gpsimd.dma_start(o_d[EV : 2 * EV, tsl], oB_t)


def _build():
    nc = bacc.Bacc(
        "TRN2",
        target_bir_lowering=False,
        debug=False,
        num_devices=N_CORES,
    )
    with tile.TileContext(nc) as tc:
        _kernel_body(tc)
    nc.compile()
    return nc


def kernel(x, W, b, S):
    global LAST_RESULTS
    nc = _CACHE.get("nc")
    if nc is None:
        nc = _build()
        _CACHE["nc"] = nc

    xf = np.asarray(x, np.float32).reshape(ROWS_TOTAL, N)
    SW = (np.asarray(S, np.float32) * np.asarray(W, np.float32))
    rot = [(r - K) % N for r in range(N + 2 * K)]       # rot row -> node
    SWr = SW[rot, :]                                    # [216, 208]
    wh = np.zeros((XROWS, WPAD), NP_BF16)
    wh[0:GR, 0:G] = SWr[0:GR, 0:G]                      # G0
    wh[QP:GE, 0:G] = SWr[G : G + GR, G : 2 * G]         # G1
    wh[0:GR, G : 2 * G] = SWr[2 * G : 2 * G + GR, 2 * G : 3 * G]   # G2
    wh[QP:GE, G : 2 * G] = SWr[3 * G : 3 * G + GR, 3 * G : 4 * G]  # G3
    bfv = np.asarray(b, np.float32).reshape(N)
    bf = np.zeros((XROWS, BPAD), np.float32)
    bf[0:G, 0] = bfv[0:G]                # E pair col 0: G0 at 0:52
    bf[QP:EV, 0] = bfv[G : 2 * G]        #               G1 at 64:116
    bf[0:G, 1] = bfv[3 * G : 4 * G]      # O pair col 1: G3 at 0:52
    bf[QP:EV, 1] = bfv[2 * G : 3 * G]    #               G2 at 64:116

    in_maps = []
    for i in range(N_CORES):
        xt = xf[i * SHARD : (i + 1) * SHARD].T          # [208, SHARD] view
        xr = np.empty((N + 2 * K, SHARD), NP_BF16)      # rotated rows
        xr[0:K] = xt[N - K : N]
        xr[K : N + K] = xt
        xr[N + K :] = xt[0:K]
        xe = np.zeros((XROWS, SHARD), NP_BF16)
        xe[0:GR] = xr[0:GR]                             # G0 rows
        xe[QP:GE] = xr[G : G + GR]                      # G1 rows
        xo = np.zeros((XROWS, SHARD), NP_BF16)
        xo[0:GR] = xr[2 * G : 2 * G + GR]               # G2 rows
        xo[QP:GE] = xr[3 * G : 3 * G + GR]              # G3 rows
        in_maps.append({"xe": xe, "xo": xo, "wh": wh, "bias": bf})
    res = run_bass_kernel_spmd(nc, in_maps, core_ids=list(range(N_CORES)))
    LAST_RESULTS = res
    out = np.empty((ROWS_TOTAL, N), np.float32)
    for i, r in enumerate(res.results):
        yt = r["outt"]                                  # [232, SHARD] bf16
        sl = slice(i * SHARD, (i + 1) * SHARD)
        out[sl, 0:G] = yt[0:G].T                        # G0
        out[sl, G : 2 * G] = yt[QP:EV].T                # G1
        out[sl, 3 * G : 4 * G] = yt[EV : EV + G].T      # G3
        out[sl, 2 * G : 3 * G] = yt[EV + QP : 2 * EV].T # G2
    return out.reshape(B, T, N)
